# revision 6
# baseline (speedup 1.0000x reference)
"""Trainium2 Bass kernel for BasicCNN+LSTM (conv3x3+ReLU+GAP -> custom LSTM scan).

Self-contained: hardcodes shapes/sharding. Data-parallel over batch B=8 across
8 NeuronCores; each core processes one batch element end-to-end, the host
gathers the 8 [1,32] results.

Per-core device pipeline (per frame t of 24):
  - DMA a host-prepacked "stack" [13, 56*336] bf16 into an SBUF row-band
    (4 round-robin bands at partitions 32s..32s+13 -> 4 concurrent PE
    row-group streams).
  - Conv as 3 accumulating matmuls (contraction over channel c via stride-3
    free-dim offsets; K=13 = 4 window rows x 3 dx-shifts + ones row for the
    conv bias). Stationary [13, 96] (96 = 2 vertically-packed pixels x 48
    filters). 14 PSUM tiles [96, 448] per frame.
  - Fused ReLU+GAP: ScalarE activation(Relu, accum_out) / VectorE
    tensor_tensor_reduce(max,add) split 8/6 across the 14 tiles.
  - Tiny LSTM-ish scan step on-chip ([1,96] gates in free layout; the
    reference's state-order swap bug is reproduced faithfully).
"""
import sys
if '/opt/trn_rl_repo' not in sys.path:
    sys.path.insert(0, '/opt/trn_rl_repo')

import numpy as np
import ml_dtypes

import concourse.bass as bass
import concourse.mybir as mybir
import concourse.tile as tile
from concourse.vector_clock import ScopedClock
from concourse.bass_utils import run_bass_kernel_spmd

# ---------------------------------------------------------------- constants
B, T, H, W, C, F, U = 8, 24, 112, 112, 3, 48, 32
JA = 56            # vertical pixel-pair blocks (112 rows / 2)
KP = 13            # stack partitions: 12 = 4 window-rows x 3 dx + 1 ones-row
M = 96             # 2 pixels x 48 filters
NSB = 14           # superblocks (PSUM tiles) per frame
NQ = 448           # columns per superblock = 4 ja-blocks x 112 w
FREE = JA * 336    # stack free size per partition (elements)

FP32 = mybir.dt.float32
BF16 = mybir.dt.bfloat16

# Which superblocks go to ScalarE (ReLU+accum) vs VectorE (TTR): 8/6 split.
ACT_QS = {0, 2, 4, 6, 8, 10, 12, 13}

LAST_RESULTS = None  # BassKernelResults of the most recent run (for test.py)

# ------------------------------------------------- TileContext drain patch
# The container's walrus rejects >1 semaphore wait per instruction; Tile's
# kernel-tail drain aggregates all end-of-kernel waits onto one Drain.
# Spread them across single-wait NOPs on the sync engine instead.
def _patched_drain_and_barrier(self, tick_clock, wait_clock):
    nc = self.nc
    probe = nc.sync.nop(nofuse=True, hint="tail_waits")
    wait_clock.add_sem_waits(probe.ins, ScopedClock({None: tick_clock.global_clock}))
    waits = list(probe.ins.sync_info.on_wait or [])
    if len(waits) > 1:
        probe.ins.sync_info.on_wait = waits[:1]
        for i in range(1, len(waits)):
            extra = nc.sync.nop(nofuse=True, hint=f"tail_waits_{i}")
            si = extra.ins.sync_info
            if si is None:
                extra.ins.sync_info = mybir.SyncInfo(on_wait=[waits[i]], on_update=[])
            else:
                si.on_wait = [waits[i]]
    nc.sync.drain()
    nc.all_engine_barrier()
    popped = nc._tile_sem_poison_stack.pop()
    assert popped is self._sem_poison
    nc.clear_and_free_semaphores(list(self.sems.allocated().values()))
    nc.all_engine_barrier()


tile.TileContext._drain_and_barrier = _patched_drain_and_barrier

# Same walrus restriction for regular instructions: spill extra sem waits
# onto preceding same-engine NOPs at commit time.
_orig_commit = tile.TileContext._commit_instruction


def _patched_commit(self, inst, *args, **kwargs):
    si = getattr(inst, 'sync_info', None)
    if si is not None and si.on_wait and len(si.on_wait) > 1 \
            and inst.engine != mybir.EngineType.Unassigned:
        waits = list(si.on_wait)
        si.on_wait = waits[-1:]
        for w in waits[:-1]:
            nop = mybir.InstNoOp(
                name=self.nc.get_next_instruction_name(),
                ins=[], outs=[], bass_is_fusable=False)
            nop.engine = inst.engine
            nop.sync_info = mybir.SyncInfo(on_wait=[w], on_update=[])
            _orig_commit(self, nop, *args, **kwargs)
    return _orig_commit(self, inst, *args, **kwargs)


tile.TileContext._commit_instruction = _patched_commit


# ------------------------------------------------------------- device code
def _build_bass():
    nc = bass.Bass('TRN2', target_bir_lowering=False, debug=False)

    xin = nc.dram_tensor('xin', [T, KP, FREE], BF16, kind='ExternalInput')
    smat_d = nc.dram_tensor('smat', [3, KP, M], BF16, kind='ExternalInput')
    wfeat_d = nc.dram_tensor('wfeat', [M, 96], FP32, kind='ExternalInput')
    whid_d = nc.dram_tensor('whid', [U, 96], FP32, kind='ExternalInput')
    gbias_d = nc.dram_tensor('gbias', [1, 96], FP32, kind='ExternalInput')
    outh_d = nc.dram_tensor('outh', [1, U], FP32, kind='ExternalOutput')

    Relu = mybir.ActivationFunctionType.Relu
    Sigmoid = mybir.ActivationFunctionType.Sigmoid
    Tanh = mybir.ActivationFunctionType.Tanh
    Amax = mybir.AluOpType.max
    Aadd = mybir.AluOpType.add

    with tile.TileContext(nc) as tc:
        const = tc.alloc_tile_pool(name='const', bufs=1)
        state = tc.alloc_tile_pool(name='state', bufs=1)
        stackp = tc.alloc_tile_pool(name='stack', bufs=2)
        psum = tc.alloc_tile_pool(name='psum', bufs=7, space='PSUM')
        spsum = tc.alloc_tile_pool(name='spsum', bufs=1, space='PSUM')
        gs = tc.alloc_tile_pool(name='gs', bufs=6)
        fs = tc.alloc_tile_pool(name='fs', bufs=6)
        ga_pool = tc.alloc_tile_pool(name='ga', bufs=4)
        tmp = tc.alloc_tile_pool(name='tmp', bufs=6)

        # constants
        sc_all = const.tile([128, 3 * M], BF16, tag='sc')
        for s in range(4):
            for c in range(3):
                nc.sync.dma_start(sc_all[32 * s:32 * s + KP, M * c:M * (c + 1)],
                                  smat_d[c])
        wfeat = const.tile([M, 96], FP32, tag='wf')
        nc.sync.dma_start(wfeat[:], wfeat_d[:])
        whid = const.tile([U, 96], FP32, tag='wh')
        nc.sync.dma_start(whid[:], whid_d[:])
        gbias = const.tile([1, 96], FP32, tag='gb')
        nc.sync.dma_start(gbias[:], gbias_d[:])
        ident = const.tile([1, 1], FP32, tag='id')
        nc.vector.memset(ident[:], 1.0)
        zeros448 = const.tile([M, NQ], FP32, tag='z448')
        nc.vector.memset(zeros448[:], 0.0)

        # persistent scan state
        cellv = state.tile([1, U], FP32, tag='cell')   # prev new_cell
        hidv = state.tile([1, U], FP32, tag='hid')     # prev new_hidden
        cell_part = state.tile([U, 1], FP32, tag='cp')  # new_cell, transposed
        nc.vector.memset(cellv[:], 0.0)
        nc.vector.memset(hidv[:], 0.0)
        nc.vector.memset(cell_part[:], 0.0)

        round_tile = None
        for t in range(T):
            s = t % 4
            if s == 0:
                round_tile = stackp.tile([128, FREE], BF16, tag='stk')
            band = round_tile[32 * s:32 * s + KP, :]
            nc.sync.dma_start(band, xin[t])
            stk4 = band.rearrange("p (j w c) -> p j w c", w=W, c=C)

            gsum = gs.tile([M, NSB], FP32, tag='gsum')
            for q in range(NSB):
                ps = psum.tile([M, NQ], FP32, tag='ps')
                ps3 = ps.rearrange("p (j w) -> p j w", w=W)
                for c in range(3):
                    rhs = stk4[:, 4 * q:4 * q + 4, :, c]
                    lhsT = sc_all[32 * s:32 * s + KP, M * c:M * (c + 1)]
                    nc.tensor.matmul(ps3[:, :, :], lhsT, rhs,
                                     start=(c == 0), stop=(c == 2),
                                     tile_position=(32 * s, 0))
                if q in ACT_QS:
                    nc.scalar.activation(ps[:], ps[:], Relu,
                                         accum_out=gsum[:, q:q + 1])
                else:
                    nc.vector.scalar_tensor_tensor(
                        out=ps[:], in0=ps[:], scalar=0.0, in1=zeros448[:],
                        op0=Amax, op1=Aadd,
                        accum_out=gsum[:, q:q + 1])

            fsum = fs.tile([M, 1], FP32, tag='fsum')
            nc.vector.reduce_sum(fsum[:], gsum[:], axis=mybir.AxisListType.X)

            # ---- scan step t (z-hidden part = prev new_cell: swap bug) ----
            pg = spsum.tile([1, 96], FP32, tag='sps')
            nc.tensor.matmul(pg[:], fsum[:], wfeat[:], start=True, stop=False)
            nc.tensor.matmul(pg[:], cell_part[:], whid[:], start=False, stop=True)
            gpre = ga_pool.tile([1, 96], FP32, tag='gpre')
            nc.vector.tensor_add(gpre[:], pg[:], gbias[:])
            ga = ga_pool.tile([1, 96], FP32, tag='ga')
            nc.scalar.activation(ga[:, 0:2 * U], gpre[:, 0:2 * U], Sigmoid)
            nc.scalar.activation(ga[:, 2 * U:3 * U], gpre[:, 2 * U:3 * U], Tanh)
            t1 = tmp.tile([1, U], FP32, tag='t1')
            nc.vector.tensor_mul(t1[:], ga[:, 0:U], hidv[:])       # sig1*prev_hid
            t2 = tmp.tile([1, U], FP32, tag='t2')
            nc.vector.tensor_mul(t2[:], ga[:, U:2 * U], ga[:, 2 * U:3 * U])
            nc.vector.tensor_add(cellv[:], t1[:], t2[:])           # new_cell
            t3 = tmp.tile([1, U], FP32, tag='t3')
            nc.scalar.activation(t3[:], cellv[:], Tanh)
            nc.vector.tensor_mul(hidv[:], cellv[:], t3[:])         # new_hidden
            if t < T - 1:
                ph = spsum.tile([U, 1], FP32, tag='sps')
                nc.tensor.transpose(ph[:], cellv[:], ident[:])
                nc.vector.tensor_copy(cell_part[:], ph[:])

        nc.sync.dma_start(outh_d[:], hidv[:])

        for p in (tmp, ga_pool, fs, gs, spsum, psum, stackp, state, const):
            p.release()

    return nc


# -------------------------------------------------------------- host prep
def _prep_inputs(x, conv_w, conv_b, W1, b1, W2, b2, W3, b3):
    x = np.asarray(x, np.float32)
    conv_w = np.asarray(conv_w, np.float32)
    conv_b = np.asarray(conv_b, np.float32)

    xp = np.zeros((B, T, H + 2, W + 2, C), np.float32)
    xp[:, :, 1:H + 1, 1:W + 1, :] = x
    xin_exp = np.empty((B, T, KP, JA, 336), np.float32)
    rows = 2 * np.arange(JA)
    for dx in range(3):
        for r in range(4):
            k = dx * 4 + r
            xin_exp[:, :, k] = xp[:, :, rows + r, dx:dx + W, :].reshape(B, T, JA, 336)
    xin_exp[:, :, 12] = 1.0
    xin_exp = xin_exp.reshape(B, T, KP, FREE).astype(ml_dtypes.bfloat16)

    smat = np.zeros((3, KP, M), np.float32)
    for c in range(3):
        for dx in range(3):
            for r in range(4):
                k = dx * 4 + r
                for i in range(2):
                    dy = r - i
                    if 0 <= dy <= 2:
                        smat[c, k, i * F:(i + 1) * F] = conv_w[dy, dx, c, :]
        if c == 1:
            smat[c, 12, 0:F] = conv_b
            smat[c, 12, F:2 * F] = conv_b
    smat = smat.astype(ml_dtypes.bfloat16)

    wfeat = np.zeros((M, 96), np.float32)
    whid = np.zeros((U, 96), np.float32)
    for g, Wg in enumerate([W1, W2, W3]):
        Wg = np.asarray(Wg, np.float32)
        for i in range(2):
            wfeat[i * F:(i + 1) * F, g * U:(g + 1) * U] = Wg[0:F, :] / float(H * W)
        whid[:, g * U:(g + 1) * U] = Wg[F:F + U, :]
    gbias = np.concatenate([np.asarray(b, np.float32) for b in (b1, b2, b3)])
    gbias = gbias.reshape(1, 96)

    return xin_exp, smat, wfeat, whid, gbias


# ------------------------------------------------------------------ kernel
def kernel(x, conv_w, conv_b, W1, b1, W2, b2, W3, b3, W4, b4):
    global LAST_RESULTS
    xin_exp, smat, wfeat, whid, gbias = _prep_inputs(
        x, conv_w, conv_b, W1, b1, W2, b2, W3, b3)

    nc = _build_bass()
    in_maps = [{
        'xin': np.ascontiguousarray(xin_exp[b]),
        'smat': smat,
        'wfeat': wfeat,
        'whid': whid,
        'gbias': gbias,
    } for b in range(B)]

    res = run_bass_kernel_spmd(nc, in_maps, core_ids=list(range(B)))
    LAST_RESULTS = res
    out = np.stack([res.results[b]['outh'][0] for b in range(B)], axis=0)
    return out.astype(np.float32)


# revision 9
# speedup vs baseline: 2.4025x; 2.4025x over previous
"""Trainium2 Bass kernel for BasicCNN+LSTM (conv3x3+ReLU+GAP -> custom LSTM scan).

Self-contained: hardcodes shapes/sharding. Data-parallel over batch B=8 across
8 NeuronCores; each core processes one batch element end-to-end, the host
gathers the 8 [1,32] results.

Per-core device pipeline (per frame t of 24):
  - DMA a host-prepacked, channel-deinterleaved "stack" [36, 56*112] bf16 into
    an SBUF row-band (partition p = c*12 + dx*4 + r holds plane c shifted by
    (dx-1, parity row r)); 2 round-robin bands at partitions {0, 64} -> 2
    concurrent PE row-group streams.
  - Conv as ONE K=36 matmul per PSUM tile (contraction over the full 3x3x3
    receptive field of a vertically-packed pixel pair; M=96 = 2 px x 48
    filters, N=448 = 4 ja-blocks x 112 w, contiguous rhs). 14 tiles/frame.
  - Fused ReLU(+conv-bias)+GAP: ScalarE activation(Relu, bias, accum_out) and
    VectorE tensor_scalar((x+bias) max 0, accum_out), 7/7 split with separate
    per-engine gsum tiles (avoids cross-engine WAW serialization).
  - Tiny LSTM-ish scan step on-chip ([1,96] gates in free layout; the
    reference's state-order swap bug is reproduced faithfully). Scan step t
    is emitted after frame t+2's conv to avoid FIFO head-of-line blocking.
"""
import sys
if '/opt/trn_rl_repo' not in sys.path:
    sys.path.insert(0, '/opt/trn_rl_repo')

import numpy as np
import ml_dtypes

import concourse.bass as bass
import concourse.mybir as mybir
import concourse.tile as tile
from concourse.vector_clock import ScopedClock
from concourse.bass_utils import run_bass_kernel_spmd

# ---------------------------------------------------------------- constants
B, T, H, W, C, F, U = 8, 24, 112, 112, 3, 48, 32
JA = 56            # vertical pixel-pair blocks (112 rows / 2)
KP = 36            # stack partitions: 3 c x 3 dx x 4 window rows
M = 96             # 2 pixels x 48 filters
NSB = 14           # superblocks (PSUM tiles) per frame
NQ = 448           # columns per superblock = 4 ja-blocks x 112 w
FREE = JA * W      # stack free size per partition (elements)

FP32 = mybir.dt.float32
BF16 = mybir.dt.bfloat16

LAST_RESULTS = None  # BassKernelResults of the most recent run (for test.py)

# ------------------------------------------------- TileContext drain patch
# The container's walrus rejects >1 semaphore wait per instruction; Tile's
# kernel-tail drain aggregates all end-of-kernel waits onto one Drain.
# Spread them across single-wait NOPs on the sync engine instead.
def _patched_drain_and_barrier(self, tick_clock, wait_clock):
    nc = self.nc
    probe = nc.sync.nop(nofuse=True, hint="tail_waits")
    wait_clock.add_sem_waits(probe.ins, ScopedClock({None: tick_clock.global_clock}))
    waits = list(probe.ins.sync_info.on_wait or [])
    if len(waits) > 1:
        probe.ins.sync_info.on_wait = waits[:1]
        for i in range(1, len(waits)):
            extra = nc.sync.nop(nofuse=True, hint=f"tail_waits_{i}")
            si = extra.ins.sync_info
            if si is None:
                extra.ins.sync_info = mybir.SyncInfo(on_wait=[waits[i]], on_update=[])
            else:
                si.on_wait = [waits[i]]
    nc.sync.drain()
    nc.all_engine_barrier()
    popped = nc._tile_sem_poison_stack.pop()
    assert popped is self._sem_poison
    nc.clear_and_free_semaphores(list(self.sems.allocated().values()))
    nc.all_engine_barrier()


tile.TileContext._drain_and_barrier = _patched_drain_and_barrier

# Same walrus restriction for regular instructions: spill extra sem waits
# onto preceding same-engine NOPs at commit time.
_orig_commit = tile.TileContext._commit_instruction


def _patched_commit(self, inst, *args, **kwargs):
    si = getattr(inst, 'sync_info', None)
    if si is not None and si.on_wait and len(si.on_wait) > 1 \
            and inst.engine != mybir.EngineType.Unassigned:
        waits = list(si.on_wait)
        si.on_wait = waits[-1:]
        for w in waits[:-1]:
            nop = mybir.InstNoOp(
                name=self.nc.get_next_instruction_name(),
                ins=[], outs=[], bass_is_fusable=False)
            nop.engine = inst.engine
            nop.sync_info = mybir.SyncInfo(on_wait=[w], on_update=[])
            _orig_commit(self, nop, *args, **kwargs)
    return _orig_commit(self, inst, *args, **kwargs)


tile.TileContext._commit_instruction = _patched_commit


# ------------------------------------------------------------- device code
def _build_bass():
    nc = bass.Bass('TRN2', target_bir_lowering=False, debug=False)

    xin = nc.dram_tensor('xin', [T, KP, FREE], BF16, kind='ExternalInput')
    smat_d = nc.dram_tensor('smat', [KP, M], BF16, kind='ExternalInput')
    cbias_d = nc.dram_tensor('cbias', [M, 1], FP32, kind='ExternalInput')
    wfeat_d = nc.dram_tensor('wfeat', [M, 96], FP32, kind='ExternalInput')
    whid_d = nc.dram_tensor('whid', [U, 96], FP32, kind='ExternalInput')
    gbias_d = nc.dram_tensor('gbias', [1, 96], FP32, kind='ExternalInput')
    outh_d = nc.dram_tensor('outh', [1, U], FP32, kind='ExternalOutput')

    Relu = mybir.ActivationFunctionType.Relu
    Sigmoid = mybir.ActivationFunctionType.Sigmoid
    Tanh = mybir.ActivationFunctionType.Tanh
    Amax = mybir.AluOpType.max
    Aadd = mybir.AluOpType.add

    with tile.TileContext(nc) as tc:
        const = tc.alloc_tile_pool(name='const', bufs=1)
        state = tc.alloc_tile_pool(name='state', bufs=1)
        stackp = tc.alloc_tile_pool(name='stack', bufs=2)
        psum = tc.alloc_tile_pool(name='psum', bufs=7, space='PSUM')
        spsum = tc.alloc_tile_pool(name='spsum', bufs=1, space='PSUM')
        gs = tc.alloc_tile_pool(name='gs', bufs=6)
        fs = tc.alloc_tile_pool(name='fs', bufs=6)
        ga_pool = tc.alloc_tile_pool(name='ga', bufs=4)
        tmp = tc.alloc_tile_pool(name='tmp', bufs=6)

        # constants
        sc_all = const.tile([128, M], BF16, tag='sc')
        for s in range(2):
            nc.sync.dma_start(sc_all[64 * s:64 * s + KP, :], smat_d[:])
        cbias = const.tile([M, 1], FP32, tag='cb')
        nc.sync.dma_start(cbias[:], cbias_d[:])
        wfeat = const.tile([M, 96], FP32, tag='wf')
        nc.sync.dma_start(wfeat[:], wfeat_d[:])
        whid = const.tile([U, 96], FP32, tag='wh')
        nc.sync.dma_start(whid[:], whid_d[:])
        gbias = const.tile([1, 96], FP32, tag='gb')
        nc.sync.dma_start(gbias[:], gbias_d[:])
        ident = const.tile([1, 1], FP32, tag='id')
        nc.vector.memset(ident[:], 1.0)
        zeros448 = const.tile([M, NQ], FP32, tag='z448')
        nc.vector.memset(zeros448[:], 0.0)

        # persistent scan state
        cellv = state.tile([1, U], FP32, tag='cell')    # prev new_cell
        hidv = state.tile([1, U], FP32, tag='hid')      # prev new_hidden
        cell_part = state.tile([U, 1], FP32, tag='cp')  # new_cell, transposed
        nc.vector.memset(cellv[:], 0.0)
        nc.vector.memset(hidv[:], 0.0)
        nc.vector.memset(cell_part[:], 0.0)

        fsums = [None] * T

        def emit_conv(t):
            s = t % 2
            if s == 0:
                emit_conv.round_tile = stackp.tile([128, FREE], BF16, tag='stk')
            band = emit_conv.round_tile[64 * s:64 * s + KP, :]
            nc.sync.dma_start(band, xin[t])
            stk3 = band.rearrange("p (j w) -> p j w", w=W)
            lhsT = sc_all[64 * s:64 * s + KP, :]

            gsumA = gs.tile([M, NSB // 2], FP32, tag='gsumA')
            gsumB = gs.tile([M, NSB // 2], FP32, tag='gsumB')
            for q in range(NSB):
                ps = psum.tile([M, NQ], FP32, tag='ps')
                ps3 = ps.rearrange("p (j w) -> p j w", w=W)
                nc.tensor.matmul(ps3[:, :, :], lhsT, stk3[:, 4 * q:4 * q + 4, :],
                                 start=True, stop=True,
                                 tile_position=(64 * s, 0))
                if q % 2 == 0:
                    nc.scalar.activation(ps[:], ps[:], Relu, bias=cbias[:],
                                         accum_out=gsumA[:, q // 2:q // 2 + 1])
                else:
                    nc.vector.scalar_tensor_tensor(
                        out=ps[:], in0=ps[:], scalar=cbias[:], in1=zeros448[:],
                        op0=Aadd, op1=Amax,
                        accum_out=gsumB[:, q // 2:q // 2 + 1])

            fsA = tmp.tile([M, 1], FP32, tag='fsA')
            nc.vector.reduce_sum(fsA[:], gsumA[:], axis=mybir.AxisListType.X)
            fsB = tmp.tile([M, 1], FP32, tag='fsB')
            nc.vector.reduce_sum(fsB[:], gsumB[:], axis=mybir.AxisListType.X)
            fsum = fs.tile([M, 1], FP32, tag='fsum')
            nc.vector.tensor_add(fsum[:], fsA[:], fsB[:])
            fsums[t] = fsum

        def emit_scan(t):
            # z-hidden part = prev new_cell (reference's state-order swap bug)
            fsum = fsums[t]
            pg = spsum.tile([1, 96], FP32, tag='sps')
            nc.tensor.matmul(pg[:], fsum[:], wfeat[:], start=True, stop=False)
            nc.tensor.matmul(pg[:], cell_part[:], whid[:], start=False, stop=True)
            gpre = ga_pool.tile([1, 96], FP32, tag='gpre')
            nc.vector.tensor_add(gpre[:], pg[:], gbias[:])
            ga = ga_pool.tile([1, 96], FP32, tag='ga')
            nc.scalar.activation(ga[:, 0:2 * U], gpre[:, 0:2 * U], Sigmoid)
            nc.scalar.activation(ga[:, 2 * U:3 * U], gpre[:, 2 * U:3 * U], Tanh)
            t1 = tmp.tile([1, U], FP32, tag='t1')
            nc.vector.tensor_mul(t1[:], ga[:, 0:U], hidv[:])       # sig1*prev_hid
            t2 = tmp.tile([1, U], FP32, tag='t2')
            nc.vector.tensor_mul(t2[:], ga[:, U:2 * U], ga[:, 2 * U:3 * U])
            nc.vector.tensor_add(cellv[:], t1[:], t2[:])           # new_cell
            t3 = tmp.tile([1, U], FP32, tag='t3')
            nc.scalar.activation(t3[:], cellv[:], Tanh)
            nc.vector.tensor_mul(hidv[:], cellv[:], t3[:])         # new_hidden
            if t < T - 1:
                ph = spsum.tile([U, 1], FP32, tag='sps')
                nc.tensor.transpose(ph[:], cellv[:], ident[:])
                nc.vector.tensor_copy(cell_part[:], ph[:])

        for t in range(T):
            emit_conv(t)
            if t >= 2:
                emit_scan(t - 2)
        emit_scan(T - 2)
        emit_scan(T - 1)

        nc.sync.dma_start(outh_d[:], hidv[:])

        for p in (tmp, ga_pool, fs, gs, spsum, psum, stackp, state, const):
            p.release()

    return nc


# -------------------------------------------------------------- host prep
def _prep_inputs(x, conv_w, conv_b, W1, b1, W2, b2, W3, b3):
    x = np.asarray(x, np.float32)
    conv_w = np.asarray(conv_w, np.float32)
    conv_b = np.asarray(conv_b, np.float32)

    xp = np.zeros((B, T, H + 2, W + 2, C), np.float32)
    xp[:, :, 1:H + 1, 1:W + 1, :] = x
    xin2 = np.empty((B, T, KP, JA, W), np.float32)
    rows = 2 * np.arange(JA)
    for c in range(3):
        for dx in range(3):
            for r in range(4):
                p = c * 12 + dx * 4 + r
                xin2[:, :, p] = np.moveaxis(
                    xp[:, :, rows + r, dx:dx + W, c], 0, 2)
    xin2 = xin2.reshape(B, T, KP, FREE).astype(ml_dtypes.bfloat16)

    smat = np.zeros((KP, M), np.float32)
    for c in range(3):
        for dx in range(3):
            for r in range(4):
                p = c * 12 + dx * 4 + r
                for i in range(2):
                    dy = r - i
                    if 0 <= dy <= 2:
                        smat[p, i * F:(i + 1) * F] = conv_w[dy, dx, c, :]
    smat = smat.astype(ml_dtypes.bfloat16)
    cbias = np.concatenate([conv_b, conv_b]).reshape(M, 1).astype(np.float32)

    wfeat = np.zeros((M, 96), np.float32)
    whid = np.zeros((U, 96), np.float32)
    for g, Wg in enumerate([W1, W2, W3]):
        Wg = np.asarray(Wg, np.float32)
        for i in range(2):
            wfeat[i * F:(i + 1) * F, g * U:(g + 1) * U] = Wg[0:F, :] / float(H * W)
        whid[:, g * U:(g + 1) * U] = Wg[F:F + U, :]
    gbias = np.concatenate([np.asarray(b, np.float32) for b in (b1, b2, b3)])
    gbias = gbias.reshape(1, 96)

    return xin2, smat, cbias, wfeat, whid, gbias


# ------------------------------------------------------------------ kernel
def kernel(x, conv_w, conv_b, W1, b1, W2, b2, W3, b3, W4, b4):
    global LAST_RESULTS
    xin2, smat, cbias, wfeat, whid, gbias = _prep_inputs(
        x, conv_w, conv_b, W1, b1, W2, b2, W3, b3)

    nc = _build_bass()
    in_maps = [{
        'xin': np.ascontiguousarray(xin2[b]),
        'smat': smat,
        'cbias': cbias,
        'wfeat': wfeat,
        'whid': whid,
        'gbias': gbias,
    } for b in range(B)]

    res = run_bass_kernel_spmd(nc, in_maps, core_ids=list(range(B)))
    LAST_RESULTS = res
    out = np.stack([res.results[b]['outh'][0] for b in range(B)], axis=0)
    return out.astype(np.float32)
